# revision 20
# baseline (speedup 1.0000x reference)
"""Trainium2 Bass kernel for nn_MultiHeadAttention_86457691669080.

Sharding: (batch, head-group) over 8 cores — core c handles batch c//2 and
heads (c%2)*8..(c%2)*8+8.  Each core runs the full pipeline for its shard in
"transposed" layout (feature dim on partitions, sequence on the free dim).

Head-PAIR attention units exploit PE tile_position packing:

  P1: Q^T/K^T projections ([dq, n], bf16; PSUM drain + bias-add on DVE),
      V in natural layout [128, kt, h, 64].  Band 0 (+V) runs upfront;
      bands 1-3 ride inside P2 pair-units on a dedicated PSUM bank.
  P2: per (pt, qh) PAIR of heads (hA=2pt rows 0-63, hB=2pt+1 rows 64-127)
      and key-tile kt:
        - adj preload: bf16 identity matmul adds adj/NORM into each scores
          PSUM (psA, psB) — full-array, FWL weight loads.
        - scores: TWO K=64 row-tiled matmuls (rows 0-1 for A, 2-3 for B)
          run CONCURRENTLY in the PE array into the two banks.
        - E = exp(NORM*PSUM) off ACT per head; Em = E*mask on DVE (A) and
          Pool (B).
        - AV: col-tiled pairs — vpA -> glAB[0:64], vpB -> glAB[64:128]
          run concurrently (E pass into cols 0:512, Em pass into 512:1024).
        - denominators: FOUR concurrent M=32 col-tiles (all-ones stationary)
          write replicated row-blocks {AE,AEm,BE,BEm} at partitions
          {0,32,64,96} of one denom bank.
      Tail per pair: ONE full-width Ln + Exp(-x) on ACT turns the whole
      denom bank into reciprocals (replicated blocks keep every partition
      initialized), a 4-row DMA gather + 4 broadcast DMAs on the idle sync
      queue build rbc/rbcm [128,512], then tmp = rho*(G*rbc) + (L*rbcm) in
      full-width DVE ops; rho = sigmoid-gate odds a/(1-a) (the global (1-a)
      cancels in the downstream L2 normalization).
  P3: signed-sqrt (|x| via mantissa mask, exp(0.5 ln|x|) on ACT) and L2
      normalization over the sequence axis, interleaved per partition-tile.
      1/||.|| folds into Wo's rows (wos) so y never waits on the norm chain.
  P4: output projection against Wo[:, group]^T; host sums the two partial
      products per batch and adds bo.  PSUM->SBUF copies and the output DMA
      are spread across engines/queues.

PSUM budget: scores(2) + glAB(2x2) + denom(1) + proj(1) = 8 banks.
Inputs stream in 128-row chunks round-robined over three DMA queues in
consumption order.  All matmuls are bf16 (fp8 in the Q/K path measured
~3e-2 output error previously; bf16 adj is both faster to load than the
old fp8 DoubleRow hi/lo trick and more precise).
"""

import numpy as np
import ml_dtypes

import concourse.bass as bass
import concourse.mybir as mybir
import concourse.tile as tile
from concourse import bacc
from concourse.bass_utils import run_bass_kernel_spmd

AF = mybir.ActivationFunctionType
ALU = mybir.AluOpType
BF16 = mybir.dt.bfloat16
F32 = mybir.dt.float32

B, N, D = 4, 1024, 1024
H = 16
HD = 64
NORM = 1.0 / np.sqrt(1024.0)
HL = 8          # heads per core
DQL = 512       # local projection width (8 heads * 64)
NCORES = 8
LAM = float(NORM)                 # exp scale
ASCALE = float(1.0 / LAM)         # adj payload prescale (= 32)

_CACHE = {}
TRACE = False  # set by test harness to collect an NTFF profile

# Restrict the activation-table-load pass to the single set that covers
# every ACT function used here (Exp/Ln/Copy).  Indices must be preserved
# (act_func_set_id indexes the full act_info.json list), so unwanted sets
# are emptied rather than removed.
_ACT_SETS_KEEP = {"natural_log_exp_and_others"}
_orig_get_activation_tables = None


def _patched_get_activation_tables(arch):
    t = _orig_get_activation_tables(arch)
    return {k: (v if k in _ACT_SETS_KEEP else set()) for k, v in t.items()}


def _install_act_table_patch():
    global _orig_get_activation_tables
    if _orig_get_activation_tables is None:
        import concourse.bacc as _bacc_mod
        _orig_get_activation_tables = _bacc_mod.get_activation_tables
        _bacc_mod.get_activation_tables = _patched_get_activation_tables


DEBUG = False


def _build(rho: float):
    _install_act_table_patch()
    nc = bacc.Bacc()
    dbg = {}
    if DEBUG:
        for nm, w, dt in (("dbg_qt", 4096, BF16), ("dbg_kt", 4096, BF16),
                          ("dbg_vp", 4096, BF16), ("dbg_e", 1024, BF16),
                          ("dbg_lns", 512, F32), ("dbg_rbc", 1024, BF16),
                          ("dbg_tmp", 4096, BF16), ("dbg_y", 4096, BF16),
                          ("dbg_wos", 4096, BF16), ("dbg_w", 1024, BF16)):
            dbg[nm] = nc.declare_dram_parameter(nm, [128, w], dt, isOutput=True)
    xq_p = nc.declare_dram_parameter("xq", [D, N], BF16, isOutput=False)
    xk_p = nc.declare_dram_parameter("xk", [D, N], BF16, isOutput=False)
    xv_p = nc.declare_dram_parameter("xv", [D, N], BF16, isOutput=False)
    wq_p = nc.declare_dram_parameter("wq", [D, DQL], BF16, isOutput=False)
    wk_p = nc.declare_dram_parameter("wk", [D, DQL], BF16, isOutput=False)
    wv_p = nc.declare_dram_parameter("wv", [D, DQL], BF16, isOutput=False)
    bq_p = nc.declare_dram_parameter("bq", [128, 4], F32, isOutput=False)
    bk_p = nc.declare_dram_parameter("bk", [128, 4], F32, isOutput=False)
    bv_p = nc.declare_dram_parameter("bv", [1, DQL], F32, isOutput=False)
    adt_p = nc.declare_dram_parameter("adt", [N, N], BF16, isOutput=False)
    mt_p = nc.declare_dram_parameter("mt", [N, N], BF16, isOutput=False)
    id_p = nc.declare_dram_parameter("idb", [128, 128], BF16, isOutput=False)
    wo_p = nc.declare_dram_parameter("wo", [DQL, D], BF16, isOutput=False)
    out_p = nc.declare_dram_parameter("out", [D, N], F32, isOutput=True)
    r_dram = nc.dram_tensor("r_scratch", [32, 512], BF16)

    with tile.TileContext(nc) as tc:
      with tc.tile_pool(name="singles", bufs=1) as singles:
        # ---- resident SBUF tensors ----
        bq_sb = singles.tile([128, 4], F32)
        bk_sb = singles.tile([128, 4], F32)
        bvb_sb = singles.tile([128, DQL], F32)
        adt_sb = singles.tile([128, 8, N], BF16)
        mt_sb = singles.tile([128, 8, N], BF16)
        id_sb = singles.tile([128, 128], BF16)
        ones_sb = singles.tile([128, 32], BF16)
        wo_sb = singles.tile([128, 4, N], BF16)
        qt_sb = singles.tile([128, 4, N], BF16)
        kt_sb = singles.tile([128, 4, N], BF16)
        vp_sb = singles.tile([128, 8, HL, 64], BF16)
        xq_sb = singles.tile([128, 8, N], BF16)
        xk_sb = singles.tile([128, 8, N], BF16)
        xv_sb = singles.tile([128, 8, N], BF16)
        wq_sb = singles.tile([128, 8, DQL], BF16)
        wk_sb = singles.tile([128, 8, DQL], BF16)
        wv_sb = singles.tile([128, 8, DQL], BF16)
        tmp_sb = singles.tile([128, 4, N], BF16)
        # P3's final write may alias tmp (tmp is dead by then)
        if DEBUG:
            y_sb = singles.tile([128, 4, N], BF16, name="y_dbg")
        else:
            y_sb = tmp_sb
        nrm2_sb = singles.tile([128, 4], F32)
        nrm2h_sb = singles.tile([128, 4, 2], F32)
        nrm_sb = singles.tile([128, 4], F32)
        rinv_sb = singles.tile([128, 4], F32)
        rlin_sb = singles.tile([128, 4], F32)
        wos_sb = singles.tile([128, 4, N], BF16)
        lnab_sb = singles.tile([128, 4, N], BF16)
        eps_sb = singles.tile([128, 1], F32)
        nc.vector.memset(eps_sb[:], 1e-30)
        nc.vector.memset(ones_sb[:], 1.0)

        # ---- input DMAs: consumption order, chunked over three queues ----
        dma_engs = (nc.sync, nc.scalar, nc.gpsimd)
        qi = 0

        def chunked_load(dst, param, nchunks, mode):
            nonlocal qi
            src = param.ap().rearrange("(t p) n -> p t n", p=128) \
                if mode == "p" else \
                param.ap().rearrange("(t p) n -> t p n", p=128)
            for t in range(nchunks):
                eng = dma_engs[qi % 3]
                qi += 1
                if mode == "p":
                    eng.dma_start(out=dst[:, t, :], in_=src[:, t, :])
                else:
                    eng.dma_start(out=dst[:, t, :], in_=src[t])

        nc.sync.dma_start(out=bq_sb[:], in_=bq_p.ap())
        nc.gpsimd.dma_start(out=bk_sb[:], in_=bk_p.ap())
        nc.scalar.dma_start(out=id_sb[:], in_=id_p.ap())
        bv_ap = bv_p.ap()
        nc.scalar.dma_start(
            out=bvb_sb[:],
            in_=bass.AP(tensor=bv_ap.tensor, offset=bv_ap.offset,
                        ap=[[0, 128]] + list(bv_ap.ap)[1:]),
        )
        chunked_load(wv_sb, wv_p, 8, "p")
        chunked_load(xv_sb, xv_p, 8, "p")
        chunked_load(wk_sb, wk_p, 8, "p")
        chunked_load(xk_sb, xk_p, 8, "p")
        chunked_load(wq_sb, wq_p, 8, "p")
        chunked_load(xq_sb, xq_p, 8, "p")
        chunked_load(adt_sb, adt_p, 8, "t")
        chunked_load(mt_sb, mt_p, 8, "t")
        chunked_load(wo_sb, wo_p, 4, "p")

        # ---- P1 (V + band 0 upfront) + P2 pair-units (bands 1-3 ride) ----
        with tc.tile_pool(name="s_ps", bufs=2, space="PSUM") as s_ps_pool, \
             tc.tile_pool(name="pj_ps", bufs=1, space="PSUM") as pj_ps_pool, \
             tc.tile_pool(name="gl_ps", bufs=2, space="PSUM") as gl_pool, \
             tc.tile_pool(name="dn_ps", bufs=1, space="PSUM") as dn_pool, \
             tc.tile_pool(name="ep", bufs=8) as epool, \
             tc.tile_pool(name="pln", bufs=2) as lnpool, \
             tc.tile_pool(name="prf", bufs=2) as rfpool, \
             tc.tile_pool(name="prb", bufs=4) as rbpool, \
             tc.tile_pool(name="pw", bufs=2) as wpool, \
             tc.tile_pool(name="p3", bufs=1) as p3:

            # Upfront P1 pieces round-robin across every PSUM pool (the P2
            # pools are idle until the first unit), so matmul groups and
            # drains pipeline; riding pieces use the dedicated pj bank.
            up_pools = (pj_ps_pool, s_ps_pool, gl_pool, dn_pool)
            up_i = [0]

            def up_pool():
                p = up_pools[up_i[0] % len(up_pools)]
                up_i[0] += 1
                return p

            def v_proj(nt):
                ps = up_pool().tile([128, 512], F32, name=f"pv_{nt}", tag="pjp")
                for dt in range(8):
                    nc.tensor.matmul(
                        ps[:],
                        xv_sb[:, dt, nt * 128:(nt + 1) * 128],
                        wv_sb[:, dt, :],
                        start=(dt == 0), stop=(dt == 7),
                    )
                nc.vector.tensor_tensor(
                    out=vp_sb[:, nt, :, :],
                    in0=ps[:].rearrange("p (h d) -> p h d", d=64),
                    in1=bvb_sb[:].rearrange("p (h d) -> p h d", d=64),
                    op=ALU.add,
                )

            def qk_piece(w_sb, x_sb_, b_sb, o_sb, dqt, nch, riding=True):
                # riding pieces hold a dedicated PSUM bank across their
                # matmuls + drain, which would starve the scores rotation
                pool = pj_ps_pool if riding else up_pool()
                ps = pool.tile([128, 512], F32,
                               name=f"pp_{o_sb.tensor.name}_{dqt}_{nch}",
                               tag="pjp")
                for dt in range(8):
                    nc.tensor.matmul(
                        ps[:],
                        w_sb[:, dt, dqt * 128:(dqt + 1) * 128],
                        x_sb_[:, dt, nch * 512:(nch + 1) * 512],
                        start=(dt == 0), stop=(dt == 7),
                    )
                # bias-add + PSUM drain on DVE (ACT is the P2 period-setter)
                nc.vector.tensor_scalar(
                    out=o_sb[:, dqt, nch * 512:(nch + 1) * 512], in0=ps[:],
                    scalar1=b_sb[:, dqt:dqt + 1], scalar2=None, op0=ALU.add,
                )

            def make_tail(u, pt, qh, gl, dn):
                qs = slice(qh * 512, (qh + 1) * 512)

                def tail_ln():
                    # Whole denom bank -> reciprocals in one Ln + one Exp
                    # (replicated 32-row blocks keep every partition valid).
                    lns = lnpool.tile([128, 512], F32, name=f"lns_{u}", tag="lns")
                    nc.scalar.activation(lns[:], dn[:], AF.Ln)
                    rf = rfpool.tile([128, 512], BF16, name=f"rf_{u}", tag="rf")
                    nc.scalar.activation(rf[:], lns[:], AF.Exp, scale=-1.0)
                    if DEBUG and u == 0:
                        nc.sync.dma_start(out=dbg["dbg_lns"].ap(), in_=lns[:])
                    return rf

                def tail_rest(rf):
                    # 4-row gather + 4 broadcast reads on the idle sync queue
                    # (a Pool broadcast would queue behind the mask-multiply
                    # stream and stall the PE).
                    rf_ap = rf[:]
                    rf_aps = list(rf_ap.ap)
                    # AP steps are flat elements: partition stride = ap[0][0]
                    nc.sync.dma_start(
                        out=r_dram.ap()[4 * u:4 * u + 4, :],
                        in_=bass.AP(tensor=rf_ap.tensor, offset=rf_ap.offset,
                                    ap=[[rf_aps[0][0] * 32, 4]] + rf_aps[1:]),
                    )
                    rbc = rbpool.tile([128, 512], BF16, name=f"rbc_{u}", tag="rbc")
                    rbcm = rbpool.tile([128, 512], BF16, name=f"rbcm_{u}", tag="rbc")
                    for dst, row in ((rbc[0:64, :], 0), (rbc[64:128, :], 2),
                                     (rbcm[0:64, :], 1), (rbcm[64:128, :], 3)):
                        rd = r_dram.ap()[4 * u + row:4 * u + row + 1, :]
                        nc.sync.dma_start(
                            out=dst,
                            in_=bass.AP(tensor=rd.tensor, offset=rd.offset,
                                        ap=[[0, 64]] + list(rd.ap)[1:]),
                        )
                    # gl is PSUM, so these stay on DVE
                    if DEBUG and u == 0:
                        nc.sync.dma_start(out=dbg["dbg_rbc"].ap()[:, 0:512], in_=rbc[:])
                        nc.sync.dma_start(out=dbg["dbg_rbc"].ap()[:, 512:1024], in_=rbcm[:])
                    wE = wpool.tile([128, 512], BF16, name=f"wE_{u}", tag="w")
                    nc.vector.tensor_tensor(out=wE[:], in0=gl[:, 0:512],
                                            in1=rbc[:], op=ALU.mult)
                    wEm = wpool.tile([128, 512], BF16, name=f"wEm_{u}", tag="w")
                    nc.vector.tensor_tensor(out=wEm[:], in0=gl[:, 512:1024],
                                            in1=rbcm[:], op=ALU.mult)
                    if DEBUG and u == 0:
                        nc.sync.dma_start(out=dbg["dbg_w"].ap()[:, 0:512], in_=wE[:])
                        nc.sync.dma_start(out=dbg["dbg_w"].ap()[:, 512:1024], in_=wEm[:])
                    nc.vector.scalar_tensor_tensor(
                        out=tmp_sb[:, pt, qs],
                        in0=wE[:], scalar=float(rho), in1=wEm[:],
                        op0=ALU.mult, op1=ALU.add,
                    )
                    # elementwise half of P3 for (pt, qh): |tmp| and its ln,
                    # plus the half-row |.| sum
                    nc.vector.tensor_reduce(
                        out=nrm2h_sb[:, pt, qh:qh + 1],
                        in_=tmp_sb[:, pt, qs],
                        axis=mybir.AxisListType.X, op=ALU.add,
                        apply_absolute_value=True,
                    )
                    abs_t = p3.tile([128, 512], BF16, name=f"abs_{u}", tag="abs")
                    nc.vector.tensor_scalar(
                        out=abs_t[:].bitcast(mybir.dt.uint16),
                        in0=tmp_sb[:, pt, qs].bitcast(mybir.dt.uint16),
                        scalar1=0x7FFF, scalar2=None, op0=ALU.bitwise_and,
                    )
                    nc.scalar.activation(lnab_sb[:, pt, qs], abs_t[:],
                                         AF.Ln, bias=eps_sb[:])
                    if qh == 1:
                        # full-row P3 finish for partition-tile pt:
                        # y = sign(tmp) * exp(0.5*ln|tmp|); 1/||.|| -> wos
                        sgn_t = p3.tile([128, N], BF16, name=f"sgn_{u}", tag="sgn")
                        nc.vector.tensor_scalar(
                            out=sgn_t[:].bitcast(mybir.dt.uint16),
                            in0=tmp_sb[:, pt, :].bitcast(mybir.dt.uint16),
                            scalar1=0x8000, scalar2=None, op0=ALU.bitwise_and,
                        )
                        nc.vector.tensor_tensor(
                            out=nrm2_sb[:, pt:pt + 1],
                            in0=nrm2h_sb[:, pt, 0:1], in1=nrm2h_sb[:, pt, 1:2],
                            op=ALU.add,
                        )
                        nc.vector.tensor_scalar_max(
                            out=nrm_sb[:, pt:pt + 1], in0=nrm2_sb[:, pt:pt + 1],
                            scalar1=1e-24,
                        )
                        nc.scalar.activation(rinv_sb[:, pt:pt + 1],
                                             nrm_sb[:, pt:pt + 1], AF.Ln)
                        nc.scalar.activation(rlin_sb[:, pt:pt + 1],
                                             rinv_sb[:, pt:pt + 1], AF.Exp,
                                             scale=-0.5)
                        nc.vector.tensor_scalar(
                            out=wos_sb[:, pt, :], in0=wo_sb[:, pt, :],
                            scalar1=rlin_sb[:, pt:pt + 1], scalar2=None,
                            op0=ALU.mult,
                        )
                        sq_t = p3.tile([128, N], BF16, name=f"sq_{u}", tag="sq")
                        nc.scalar.activation(sq_t[:], lnab_sb[:, pt, :], AF.Exp,
                                             scale=0.5)
                        nc.vector.tensor_tensor(
                            out=y_sb[:, pt, :].bitcast(mybir.dt.uint16),
                            in0=sq_t[:].bitcast(mybir.dt.uint16),
                            in1=sgn_t[:].bitcast(mybir.dt.uint16),
                            op=ALU.bitwise_or,
                        )

                return tail_ln, tail_rest

            state = {"rest": None}

            def emit_unit(u, pieces):
                pt, qh = u // 2, u % 2
                hA, hB = 2 * pt, 2 * pt + 1
                qs = slice(qh * 512, (qh + 1) * 512)
                kc = lambda kt: slice(kt * 128, (kt + 1) * 128)
                gl = gl_pool.tile([128, 1024], F32, name=f"gl_{u}", tag="gl")
                dn = dn_pool.tile([128, 512], F32, name=f"dn_{u}", tag="dn")
                for kt in range(8):
                    psA = s_ps_pool.tile([128, 512], F32,
                                         name=f"psA_{u}_{kt}", tag="sps")
                    psB = s_ps_pool.tile([128, 512], F32,
                                         name=f"psB_{u}_{kt}", tag="sps")
                    # adj preloads: bf16 identity matmuls (FWL weight loads)
                    nc.tensor.matmul(psA[:], id_sb[:], adt_sb[:, kt, qs],
                                     start=True, stop=False)
                    nc.tensor.matmul(psB[:], id_sb[:], adt_sb[:, kt, qs],
                                     start=True, stop=False)
                    # row-tiled scores: A in array rows 0-63, B in 64-127,
                    # hardware-concurrent into the two banks
                    nc.tensor.matmul(
                        psA[:], kt_sb[0:64, pt, kc(kt)], qt_sb[0:64, pt, qs],
                        start=False, stop=True, tile_position=(0, 0),
                    )
                    nc.tensor.matmul(
                        psB[:], kt_sb[64:128, pt, kc(kt)], qt_sb[64:128, pt, qs],
                        start=False, stop=True, tile_position=(64, 0),
                    )
                    # e = exp(lambda*(S' + adj/lambda)) straight off ACT
                    eA = epool.tile([128, 512], BF16, name=f"eA_{u}_{kt}", tag="e")
                    nc.scalar.activation(eA[:], psA[:], AF.Exp, scale=LAM)
                    eB = epool.tile([128, 512], BF16, name=f"eB_{u}_{kt}", tag="e")
                    nc.scalar.activation(eB[:], psB[:], AF.Exp, scale=LAM)
                    if DEBUG and u == 0 and kt == 0:
                        nc.sync.dma_start(out=dbg["dbg_e"].ap()[:, 0:512], in_=eA[:])
                        nc.sync.dma_start(out=dbg["dbg_e"].ap()[:, 512:1024], in_=eB[:])
                    emA = epool.tile([128, 512], BF16, name=f"emA_{u}_{kt}", tag="e")
                    nc.vector.tensor_tensor(out=emA[:], in0=eA[:],
                                            in1=mt_sb[:, kt, qs], op=ALU.mult)
                    emB = epool.tile([128, 512], BF16, name=f"emB_{u}_{kt}", tag="e")
                    nc.gpsimd.tensor_tensor(out=emB[:], in0=eB[:],
                                            in1=mt_sb[:, kt, qs], op=ALU.mult)
                    st, sp = (kt == 0), (kt == 7)
                    # col-tiled AV pairs: A -> partitions 0-63, B -> 64-127
                    # (group tracking is per partition-slice: each tile runs
                    # its own start/stop accumulation group)
                    # (the sim's group tracker keys on partition 0 of the
                    # zero region, so non-base-0 tiles skip the check; the
                    # pending-zero data semantics are partition-base-aware
                    # and verified correct)
                    nc.tensor.matmul(gl[0:64, 0:512], vp_sb[:, kt, hA, :], eA[:],
                                     start=st, stop=sp, tile_position=(0, 0))
                    nc.tensor.matmul(gl[64:128, 0:512], vp_sb[:, kt, hB, :], eB[:],
                                     start=st, stop=sp, tile_position=(0, 64),
                                     skip_group_check=True)
                    nc.tensor.matmul(gl[0:64, 512:1024], vp_sb[:, kt, hA, :], emA[:],
                                     start=st, stop=sp, tile_position=(0, 0))
                    nc.tensor.matmul(gl[64:128, 512:1024], vp_sb[:, kt, hB, :], emB[:],
                                     start=st, stop=sp, tile_position=(0, 64),
                                     skip_group_check=True)
                    # denominators: four concurrent M=32 col-tiles
                    nc.tensor.matmul(dn[0:32, :], ones_sb[:], eA[:],
                                     start=st, stop=sp, tile_position=(0, 0))
                    nc.tensor.matmul(dn[32:64, :], ones_sb[:], emA[:],
                                     start=st, stop=sp, tile_position=(0, 32),
                                     skip_group_check=True)
                    nc.tensor.matmul(dn[64:96, :], ones_sb[:], eB[:],
                                     start=st, stop=sp, tile_position=(0, 64),
                                     skip_group_check=True)
                    nc.tensor.matmul(dn[96:128, :], ones_sb[:], emB[:],
                                     start=st, stop=sp, tile_position=(0, 96),
                                     skip_group_check=True)
                    if kt == 1 and state["rest"] is not None:
                        # previous unit's tail body: deferred so its gl-PSUM
                        # reads and broadcast latency hide behind this stream
                        state["rest"]()
                        state["rest"] = None
                    if kt == 3 and len(pieces) > 0:
                        pieces[0]()
                    if kt == 5 and len(pieces) > 1:
                        pieces[1]()
                tail_ln, tail_rest = make_tail(u, pt, qh, gl, dn)
                # Ln/Exp issue now (drains + frees the denom bank early);
                # the DMA broadcast + DVE body defers into the next unit.
                rf = tail_ln()
                state["rest"] = lambda: tail_rest(rf)

            # V projections first (vp is needed by every unit's AV matmuls),
            # then band 0 Q/K.
            for nt in range(8):
                v_proj(nt)
            qk_piece(wk_sb, xk_sb, bk_sb, kt_sb, 0, 0, riding=False)
            qk_piece(wk_sb, xk_sb, bk_sb, kt_sb, 0, 1, riding=False)
            qk_piece(wq_sb, xq_sb, bq_sb, qt_sb, 0, 0, riding=False)
            qk_piece(wq_sb, xq_sb, bq_sb, qt_sb, 0, 1, riding=False)
            # pair-units; band pt+1 projection pieces ride along (2 per unit)
            for u in range(8):
                pt = u // 2
                pieces = []
                if pt < 3:
                    w_x_b_o = ((wk_sb, xk_sb, bk_sb, kt_sb),
                               (wq_sb, xq_sb, bq_sb, qt_sb))[u % 2]
                    pieces = [
                        (lambda args=w_x_b_o, d=pt + 1, n=nch:
                         qk_piece(*args, d, n)) for nch in range(2)
                    ]
                emit_unit(u, pieces)
            state["rest"]()
            if DEBUG:
                for nm, t in (("dbg_qt", qt_sb), ("dbg_kt", kt_sb),
                              ("dbg_tmp", tmp_sb), ("dbg_y", y_sb),
                              ("dbg_wos", wos_sb)):
                    nc.sync.dma_start(
                        out=dbg[nm].ap(),
                        in_=t[:].rearrange("p a n -> p (a n)"))
                nc.sync.dma_start(
                    out=dbg["dbg_vp"].ap(),
                    in_=vp_sb[:].rearrange("p a h d -> p (a h d)"))

        # ---- P4: output projection (partial; host sums pairs + bo) ----
        with tc.tile_pool(name="o_ps", bufs=8, space="PSUM") as o_ps_pool, \
             tc.tile_pool(name="oc", bufs=3) as oc_pool:
            dma_out_engs = (nc.sync, nc.scalar, nc.gpsimd)
            for dot in range(8):
                for qch in range(2):
                    i = dot * 2 + qch
                    ps = o_ps_pool.tile([128, 512], F32,
                                        name=f"ops_{dot}_{qch}", tag="ops")
                    for dvt in range(4):
                        nc.tensor.matmul(
                            ps[:],
                            wos_sb[:, dvt, dot * 128:(dot + 1) * 128],
                            y_sb[:, dvt, qch * 512:(qch + 1) * 512],
                            start=(dvt == 0), stop=(dvt == 3),
                        )
                    ot = oc_pool.tile([128, 512], F32)
                    # PSUM source: only DVE/ACT may read it
                    if i % 2 == 0:
                        nc.vector.tensor_copy(out=ot[:], in_=ps[:])
                    else:
                        nc.scalar.copy(out=ot[:], in_=ps[:])
                    dma_out_engs[i % 3].dma_start(
                        out=out_p.ap()[dot * 128:(dot + 1) * 128,
                                       qch * 512:(qch + 1) * 512],
                        in_=ot[:],
                    )

    nc.finalize()
    return nc


def _get(rho: float):
    key = round(float(rho), 9)
    if key not in _CACHE:
        _CACHE[key] = _build(key)
    return _CACHE[key]


def _make_in_maps(query, key, value, adj, mask, Wq, bq, Wk, bk, Wv, bv, Wo):
    f32 = np.float32
    bf = lambda x: np.ascontiguousarray(np.asarray(x, f32)).astype(ml_dtypes.bfloat16)

    idb = np.eye(128, dtype=f32).astype(ml_dtypes.bfloat16)

    in_maps = []
    for b in range(B):
        xqT = bf(np.asarray(query[b], f32).T)
        xkT = bf(np.asarray(key[b], f32).T)
        xvT = bf(np.asarray(value[b], f32).T)
        adt = bf(np.asarray(adj[b, 0], f32).T * np.float32(ASCALE))
        mtT = bf((np.asarray(mask[b, 0]) != 0).astype(f32).T)
        for g in range(2):
            rows = slice(g * DQL, (g + 1) * DQL)
            in_maps.append({
                "xq": xqT, "xk": xkT, "xv": xvT,
                "wq": bf(np.asarray(Wq, f32)[rows].T),
                "wk": bf(np.asarray(Wk, f32)[rows].T),
                "wv": bf(np.asarray(Wv, f32)[rows].T),
                "bq": np.ascontiguousarray(np.asarray(bq, f32)[rows].reshape(4, 128).T),
                "bk": np.ascontiguousarray(np.asarray(bk, f32)[rows].reshape(4, 128).T),
                "bv": np.ascontiguousarray(np.asarray(bv, f32)[rows].reshape(1, DQL)),
                "adt": adt, "mt": mtT, "idb": idb,
                "wo": bf(np.asarray(Wo, f32)[:, rows].T),
            })
    return in_maps


def kernel(query, key, value, adj, mask, Wq, bq, Wk, bk, Wv, bv, Wo, bo, alpha,
           _want_results=False):
    f32 = np.float32
    a = 1.0 / (1.0 + np.exp(-np.float64(np.asarray(alpha, f32)[0])))
    rho = float(a / (1.0 - a))
    nc = _get(rho)

    in_maps = _make_in_maps(query, key, value, adj, mask,
                            Wq, bq, Wk, bk, Wv, bv, Wo)

    res = run_bass_kernel_spmd(nc, in_maps, list(range(NCORES)), trace=TRACE)
    out = np.empty((B, N, D), f32)
    bo_f = np.asarray(bo, f32)
    for b in range(B):
        out[b] = (res.results[2 * b]["out"] + res.results[2 * b + 1]["out"]).T + bo_f
    if _want_results:
        return out, res
    return out


# revision 26
# speedup vs baseline: 1.4424x; 1.4424x over previous
"""Trainium2 Bass kernel for nn_MultiHeadAttention_86457691669080.

Sharding: (batch, head-group) over 8 cores — core c handles batch c//2 and
heads (c%2)*8..(c%2)*8+8.  Each core runs the full pipeline for its shard in
"transposed" layout (feature dim on partitions, sequence on the free dim).

Head-PAIR attention units exploit PE tile_position packing:

  P1: Q^T/K^T projections ([dq, n], bf16; PSUM drain + bias-add on DVE),
      V in natural layout [128, kt, h, 64].  Band 0 (+V) runs upfront;
      bands 1-3 ride inside P2 pair-units on a dedicated PSUM bank.
  P2: per (pt, qh) PAIR of heads (hA=2pt rows 0-63, hB=2pt+1 rows 64-127)
      and key-tile kt:
        - adj preload: bf16 identity matmul adds adj/NORM into each scores
          PSUM (psA, psB) — full-array, FWL weight loads.
        - scores: TWO K=64 row-tiled matmuls (rows 0-1 for A, 2-3 for B)
          run CONCURRENTLY in the PE array into the two banks.
        - E = exp(NORM*PSUM) off ACT per head; Em = E*mask on DVE (A) and
          Pool (B).
        - AV: col-tiled pairs — vpA -> glAB[0:64], vpB -> glAB[64:128]
          run concurrently (E pass into cols 0:512, Em pass into 512:1024).
        - denominators: FOUR concurrent M=32 col-tiles (all-ones stationary)
          write replicated row-blocks {AE,AEm,BE,BEm} at partitions
          {0,32,64,96} of one denom bank.
      Tail per pair: ONE full-width Ln + Exp(-x) on ACT turns the whole
      denom bank into reciprocals (replicated blocks keep every partition
      initialized), a 4-row DMA gather + 4 broadcast DMAs on the idle sync
      queue build rbc/rbcm [128,512], then tmp = rho*(G*rbc) + (L*rbcm) in
      full-width DVE ops; rho = sigmoid-gate odds a/(1-a) (the global (1-a)
      cancels in the downstream L2 normalization).
  P3: signed-sqrt (|x| via mantissa mask, exp(0.5 ln|x|) on ACT) and L2
      normalization over the sequence axis, interleaved per partition-tile.
      1/||.|| folds into Wo's rows (wos) so y never waits on the norm chain.
  P4: output projection against Wo[:, group]^T; host sums the two partial
      products per batch and adds bo.  PSUM->SBUF copies and the output DMA
      are spread across engines/queues.

PSUM budget: scores(2) + glAB(2x2) + denom(1) + proj(1) = 8 banks.
Inputs stream in 128-row chunks round-robined over three DMA queues in
consumption order.  All matmuls are bf16 (fp8 in the Q/K path measured
~3e-2 output error previously; bf16 adj is both faster to load than the
old fp8 DoubleRow hi/lo trick and more precise).
"""

import numpy as np
import ml_dtypes

import concourse.bass as bass
import concourse.mybir as mybir
import concourse.tile as tile
from concourse import bacc
from concourse.bass_utils import run_bass_kernel_spmd

AF = mybir.ActivationFunctionType
ALU = mybir.AluOpType
BF16 = mybir.dt.bfloat16
F32 = mybir.dt.float32

B, N, D = 4, 1024, 1024
H = 16
HD = 64
NORM = 1.0 / np.sqrt(1024.0)
HL = 8          # heads per core
DQL = 512       # local projection width (8 heads * 64)
NCORES = 8
LAM = float(NORM)                 # exp scale
ASCALE = float(1.0 / LAM)         # adj payload prescale (= 32)

_CACHE = {}
TRACE = False  # set by test harness to collect an NTFF profile

# Restrict the activation-table-load pass to the single set that covers
# every ACT function used here (Exp/Ln/Copy).  Indices must be preserved
# (act_func_set_id indexes the full act_info.json list), so unwanted sets
# are emptied rather than removed.
_ACT_SETS_KEEP = {"natural_log_exp_and_others"}
_orig_get_activation_tables = None


def _patched_get_activation_tables(arch):
    t = _orig_get_activation_tables(arch)
    return {k: (v if k in _ACT_SETS_KEEP else set()) for k, v in t.items()}


def _install_act_table_patch():
    global _orig_get_activation_tables
    if _orig_get_activation_tables is None:
        import concourse.bacc as _bacc_mod
        _orig_get_activation_tables = _bacc_mod.get_activation_tables
        _bacc_mod.get_activation_tables = _patched_get_activation_tables


DEBUG = False


def _build(rho: float):
    _install_act_table_patch()
    nc = bacc.Bacc()
    dbg = {}
    if DEBUG:
        for nm, w, dt in (("dbg_qt", 4096, BF16), ("dbg_kt", 4096, BF16),
                          ("dbg_vp", 4096, BF16), ("dbg_e", 1024, BF16),
                          ("dbg_lns", 512, F32), ("dbg_rbc", 1024, BF16),
                          ("dbg_tmp", 4096, BF16), ("dbg_y", 4096, BF16),
                          ("dbg_wos", 4096, BF16), ("dbg_w", 1024, BF16)):
            dbg[nm] = nc.declare_dram_parameter(nm, [128, w], dt, isOutput=True)
    xq_p = nc.declare_dram_parameter("xq", [D, N], BF16, isOutput=False)
    xk_p = nc.declare_dram_parameter("xk", [D, N], BF16, isOutput=False)
    xv_p = nc.declare_dram_parameter("xv", [D, N], BF16, isOutput=False)
    wq_p = nc.declare_dram_parameter("wq", [D, DQL], BF16, isOutput=False)
    wk_p = nc.declare_dram_parameter("wk", [D, DQL], BF16, isOutput=False)
    wv_p = nc.declare_dram_parameter("wv", [D, DQL], BF16, isOutput=False)
    bq_p = nc.declare_dram_parameter("bq", [128, 4], F32, isOutput=False)
    bk_p = nc.declare_dram_parameter("bk", [128, 4], F32, isOutput=False)
    bv_p = nc.declare_dram_parameter("bv", [1, DQL], F32, isOutput=False)
    adt_p = nc.declare_dram_parameter("adt", [N, N], BF16, isOutput=False)
    mt_p = nc.declare_dram_parameter("mt", [N, N], BF16, isOutput=False)
    id_p = nc.declare_dram_parameter("idb", [128, 128], BF16, isOutput=False)
    wo_p = nc.declare_dram_parameter("wo", [DQL, D], BF16, isOutput=False)
    out_p = nc.declare_dram_parameter("out", [D, N], F32, isOutput=True)
    r_dram = nc.dram_tensor("r_scratch", [32, 512], BF16)

    with tile.TileContext(nc) as tc:
      with tc.tile_pool(name="singles", bufs=1) as singles:
        # ---- resident SBUF tensors ----
        bq_sb = singles.tile([128, 4], F32)
        bk_sb = singles.tile([128, 4], F32)
        bvb_sb = singles.tile([128, DQL], F32)
        adt_sb = singles.tile([128, 8, N], BF16)
        mt_sb = singles.tile([128, 8, N], BF16)
        id_sb = singles.tile([128, 128], BF16)
        ones_sb = singles.tile([128, 32], BF16)
        wo_sb = singles.tile([128, 4, N], BF16)
        qt_sb = singles.tile([128, 4, N], BF16)
        kt_sb = singles.tile([128, 4, N], BF16)
        vp_sb = singles.tile([128, 8, HL, 64], BF16)
        xq_sb = singles.tile([128, 8, N], BF16)
        xk_sb = singles.tile([128, 8, N], BF16)
        xv_sb = singles.tile([128, 8, N], BF16)
        wq_sb = singles.tile([128, 8, DQL], BF16)
        wk_sb = singles.tile([128, 8, DQL], BF16)
        wv_sb = singles.tile([128, 8, DQL], BF16)
        tmp_sb = singles.tile([128, 4, N], BF16)
        # P3's final write may alias tmp (tmp is dead by then)
        if DEBUG:
            y_sb = singles.tile([128, 4, N], BF16, name="y_dbg")
        else:
            y_sb = tmp_sb
        nrm2_sb = singles.tile([128, 4], F32)
        nrm2h_sb = singles.tile([128, 4, 2], F32)
        nrm_sb = singles.tile([128, 4], F32)
        rinv_sb = singles.tile([128, 4], F32)
        rlin_sb = singles.tile([128, 4], F32)
        wos_sb = singles.tile([128, 4, N], BF16)
        lnab_sb = singles.tile([128, 4, N], BF16)
        eps_sb = singles.tile([128, 1], F32)
        nc.vector.memset(eps_sb[:], 1e-30)
        nc.vector.memset(ones_sb[:], 1.0)

        # ---- input DMAs: consumption order, chunked over three queues ----
        dma_engs = (nc.sync, nc.scalar, nc.gpsimd)
        qi = 0

        def chunked_load(dst, param, nchunks, mode):
            nonlocal qi
            src = param.ap().rearrange("(t p) n -> p t n", p=128) \
                if mode == "p" else \
                param.ap().rearrange("(t p) n -> t p n", p=128)
            for t in range(nchunks):
                eng = dma_engs[qi % 3]
                qi += 1
                if mode == "p":
                    eng.dma_start(out=dst[:, t, :], in_=src[:, t, :])
                else:
                    eng.dma_start(out=dst[:, t, :], in_=src[t])

        nc.sync.dma_start(out=bq_sb[:], in_=bq_p.ap())
        nc.gpsimd.dma_start(out=bk_sb[:], in_=bk_p.ap())
        nc.scalar.dma_start(out=id_sb[:], in_=id_p.ap())
        bv_ap = bv_p.ap()
        nc.scalar.dma_start(
            out=bvb_sb[:],
            in_=bass.AP(tensor=bv_ap.tensor, offset=bv_ap.offset,
                        ap=[[0, 128]] + list(bv_ap.ap)[1:]),
        )
        chunked_load(wv_sb, wv_p, 8, "p")
        chunked_load(xv_sb, xv_p, 8, "p")
        chunked_load(wk_sb, wk_p, 8, "p")
        chunked_load(xk_sb, xk_p, 8, "p")
        chunked_load(wq_sb, wq_p, 8, "p")
        chunked_load(xq_sb, xq_p, 8, "p")
        chunked_load(adt_sb, adt_p, 8, "t")
        chunked_load(mt_sb, mt_p, 8, "t")
        chunked_load(wo_sb, wo_p, 4, "p")

        # ---- P1 (V + band 0 upfront) + P2 pair-units (bands 1-3 ride) ----
        with tc.tile_pool(name="s_ps", bufs=2, space="PSUM") as s_ps_pool, \
             tc.tile_pool(name="pj_ps", bufs=1, space="PSUM") as pj_ps_pool, \
             tc.tile_pool(name="gl_ps", bufs=2, space="PSUM") as gl_pool, \
             tc.tile_pool(name="dn_ps", bufs=1, space="PSUM") as dn_pool, \
             tc.tile_pool(name="ep", bufs=12) as epool, \
             tc.tile_pool(name="pln", bufs=2) as lnpool, \
             tc.tile_pool(name="prf", bufs=2) as rfpool, \
             tc.tile_pool(name="prb", bufs=4) as rbpool, \
             tc.tile_pool(name="pw", bufs=2) as wpool, \
             tc.tile_pool(name="p3", bufs=1) as p3:

            # Upfront P1 pieces round-robin across every PSUM pool (the P2
            # pools are idle until the first unit), so matmul groups and
            # drains pipeline; riding pieces use the dedicated pj bank.
            up_pools = ((pj_ps_pool, "pjp"), (s_ps_pool, "sps"), (dn_pool, "dn"))
            up_i = [0]

            def up_tile(name):
                p, tg = up_pools[up_i[0] % len(up_pools)]
                up_i[0] += 1
                return p.tile([128, 512], F32, name=name, tag=tg)

            def v_proj(nt):
                ps = up_tile(f"pv_{nt}")
                for dt in range(8):
                    nc.tensor.matmul(
                        ps[:],
                        xv_sb[:, dt, nt * 128:(nt + 1) * 128],
                        wv_sb[:, dt, :],
                        start=(dt == 0), stop=(dt == 7),
                    )
                nc.vector.tensor_tensor(
                    out=vp_sb[:, nt, :, :],
                    in0=ps[:].rearrange("p (h d) -> p h d", d=64),
                    in1=bvb_sb[:].rearrange("p (h d) -> p h d", d=64),
                    op=ALU.add,
                )

            def qk_piece(w_sb, x_sb_, b_sb, o_sb, dqt, nch, riding=True):
                # riding pieces hold a dedicated PSUM bank across their
                # matmuls + drain, which would starve the scores rotation
                nm = f"pp_{o_sb.tensor.name}_{dqt}_{nch}"
                if riding:
                    ps = pj_ps_pool.tile([128, 512], F32, name=nm, tag="pjp")
                else:
                    ps = up_tile(nm)
                for dt in range(8):
                    nc.tensor.matmul(
                        ps[:],
                        w_sb[:, dt, dqt * 128:(dqt + 1) * 128],
                        x_sb_[:, dt, nch * 512:(nch + 1) * 512],
                        start=(dt == 0), stop=(dt == 7),
                    )
                # bias-add + PSUM drain on DVE (ACT is the P2 period-setter)
                nc.vector.tensor_scalar(
                    out=o_sb[:, dqt, nch * 512:(nch + 1) * 512], in0=ps[:],
                    scalar1=b_sb[:, dqt:dqt + 1], scalar2=None, op0=ALU.add,
                )

            def make_tail(u, pt, qh, gl, dn):
                qs = slice(qh * 512, (qh + 1) * 512)

                def tail_ln():
                    # Whole denom bank -> reciprocals in one Ln + one Exp
                    # (replicated 32-row blocks keep every partition valid).
                    lns = lnpool.tile([128, 512], F32, name=f"lns_{u}", tag="lns")
                    nc.scalar.activation(lns[:], dn[:], AF.Ln)
                    rf = rfpool.tile([128, 512], BF16, name=f"rf_{u}", tag="rf")
                    nc.scalar.activation(rf[:], lns[:], AF.Exp, scale=-1.0)
                    if DEBUG and u == 0:
                        nc.sync.dma_start(out=dbg["dbg_lns"].ap(), in_=lns[:])
                    return rf

                def tail_rest(rf):
                    # 4-row gather + 4 broadcast reads on the idle sync queue
                    # (a Pool broadcast would queue behind the mask-multiply
                    # stream and stall the PE).
                    rf_ap = rf[:]
                    rf_aps = list(rf_ap.ap)
                    # AP steps are flat elements: partition stride = ap[0][0]
                    nc.sync.dma_start(
                        out=r_dram.ap()[4 * u:4 * u + 4, :],
                        in_=bass.AP(tensor=rf_ap.tensor, offset=rf_ap.offset,
                                    ap=[[rf_aps[0][0] * 32, 4]] + rf_aps[1:]),
                    )
                    rbc = rbpool.tile([128, 512], BF16, name=f"rbc_{u}", tag="rbc")
                    rbcm = rbpool.tile([128, 512], BF16, name=f"rbcm_{u}", tag="rbc")
                    for dst, row in ((rbc[0:64, :], 0), (rbc[64:128, :], 2),
                                     (rbcm[0:64, :], 1), (rbcm[64:128, :], 3)):
                        rd = r_dram.ap()[4 * u + row:4 * u + row + 1, :]
                        nc.sync.dma_start(
                            out=dst,
                            in_=bass.AP(tensor=rd.tensor, offset=rd.offset,
                                        ap=[[0, 64]] + list(rd.ap)[1:]),
                        )
                    # gl is PSUM, so these stay on DVE
                    if DEBUG and u == 0:
                        nc.sync.dma_start(out=dbg["dbg_rbc"].ap()[:, 0:512], in_=rbc[:])
                        nc.sync.dma_start(out=dbg["dbg_rbc"].ap()[:, 512:1024], in_=rbcm[:])
                    wE = wpool.tile([128, 512], BF16, name=f"wE_{u}", tag="w")
                    nc.vector.tensor_tensor(out=wE[:], in0=gl[:, 0:512],
                                            in1=rbc[:], op=ALU.mult)
                    wEm = wpool.tile([128, 512], BF16, name=f"wEm_{u}", tag="w")
                    nc.vector.tensor_tensor(out=wEm[:], in0=gl[:, 512:1024],
                                            in1=rbcm[:], op=ALU.mult)
                    if DEBUG and u == 0:
                        nc.sync.dma_start(out=dbg["dbg_w"].ap()[:, 0:512], in_=wE[:])
                        nc.sync.dma_start(out=dbg["dbg_w"].ap()[:, 512:1024], in_=wEm[:])
                    nc.vector.scalar_tensor_tensor(
                        out=tmp_sb[:, pt, qs],
                        in0=wE[:], scalar=float(rho), in1=wEm[:],
                        op0=ALU.mult, op1=ALU.add,
                    )
                    # elementwise half of P3 for (pt, qh): |tmp| and its ln,
                    # plus the half-row |.| sum
                    nc.vector.tensor_reduce(
                        out=nrm2h_sb[:, pt, qh:qh + 1],
                        in_=tmp_sb[:, pt, qs],
                        axis=mybir.AxisListType.X, op=ALU.add,
                        apply_absolute_value=True,
                    )
                    abs_t = p3.tile([128, 512], BF16, name=f"abs_{u}", tag="abs")
                    nc.vector.tensor_scalar(
                        out=abs_t[:].bitcast(mybir.dt.uint16),
                        in0=tmp_sb[:, pt, qs].bitcast(mybir.dt.uint16),
                        scalar1=0x7FFF, scalar2=None, op0=ALU.bitwise_and,
                    )
                    nc.scalar.activation(lnab_sb[:, pt, qs], abs_t[:],
                                         AF.Ln, bias=eps_sb[:])
                    if qh == 1:
                        # full-row P3 finish for partition-tile pt:
                        # y = sign(tmp) * exp(0.5*ln|tmp|); 1/||.|| -> wos
                        sgn_t = p3.tile([128, N], BF16, name=f"sgn_{u}", tag="sgn")
                        nc.vector.tensor_scalar(
                            out=sgn_t[:].bitcast(mybir.dt.uint16),
                            in0=tmp_sb[:, pt, :].bitcast(mybir.dt.uint16),
                            scalar1=0x8000, scalar2=None, op0=ALU.bitwise_and,
                        )
                        nc.vector.tensor_tensor(
                            out=nrm2_sb[:, pt:pt + 1],
                            in0=nrm2h_sb[:, pt, 0:1], in1=nrm2h_sb[:, pt, 1:2],
                            op=ALU.add,
                        )
                        nc.vector.tensor_scalar_max(
                            out=nrm_sb[:, pt:pt + 1], in0=nrm2_sb[:, pt:pt + 1],
                            scalar1=1e-24,
                        )
                        nc.scalar.activation(rinv_sb[:, pt:pt + 1],
                                             nrm_sb[:, pt:pt + 1], AF.Ln)
                        nc.scalar.activation(rlin_sb[:, pt:pt + 1],
                                             rinv_sb[:, pt:pt + 1], AF.Exp,
                                             scale=-0.5)
                        nc.vector.tensor_scalar(
                            out=wos_sb[:, pt, :], in0=wo_sb[:, pt, :],
                            scalar1=rlin_sb[:, pt:pt + 1], scalar2=None,
                            op0=ALU.mult,
                        )
                        sq_t = p3.tile([128, N], BF16, name=f"sq_{u}", tag="sq")
                        nc.scalar.activation(sq_t[:], lnab_sb[:, pt, :], AF.Exp,
                                             scale=0.5)
                        nc.vector.tensor_tensor(
                            out=y_sb[:, pt, :].bitcast(mybir.dt.uint16),
                            in0=sq_t[:].bitcast(mybir.dt.uint16),
                            in1=sgn_t[:].bitcast(mybir.dt.uint16),
                            op=ALU.bitwise_or,
                        )

                return tail_ln, tail_rest

            # Global 2-step software pipeline: the AV + denominator matmuls
            # for step g issue only after step g+2's scores, when all four
            # e/em operands are long since materialized.  The PE stream is
            # then wait-free at the packing points, so the row/col-tiled
            # pairs actually run concurrently and LDWEIGHTS pull-ahead works.
            state = {"rest": None, "pending": [], "done": []}

            def run_cons():
                state["pending"].pop(0)()

            def emit_unit(u, pieces):
                pt, qh = u // 2, u % 2
                hA, hB = 2 * pt, 2 * pt + 1
                qs = slice(qh * 512, (qh + 1) * 512)
                kc = lambda kt: slice(kt * 128, (kt + 1) * 128)
                gl = gl_pool.tile([128, 1024], F32, name=f"gl_{u}", tag="gl")
                dn = dn_pool.tile([128, 512], F32, name=f"dn_{u}", tag="dn")

                def make_cons(kt, eA, eB, emA, emB):
                    st, sp = (kt == 0), (kt == 7)

                    def cons():
                        # col-tiled AV pairs: A -> partitions 0-63, B -> 64-127
                        # (each partition-slice runs its own accumulation
                        # group; the sim group tracker keys on partition 0 of
                        # the zero region, so non-base-0 tiles skip the check
                        # — pending-zero data semantics are base-aware)
                        nc.tensor.matmul(gl[0:64, 0:512], vp_sb[:, kt, hA, :],
                                         eA[:], start=st, stop=sp,
                                         tile_position=(0, 0))
                        nc.tensor.matmul(gl[64:128, 0:512], vp_sb[:, kt, hB, :],
                                         eB[:], start=st, stop=sp,
                                         tile_position=(0, 64),
                                         skip_group_check=True)
                        nc.tensor.matmul(gl[0:64, 512:1024], vp_sb[:, kt, hA, :],
                                         emA[:], start=st, stop=sp,
                                         tile_position=(0, 0))
                        nc.tensor.matmul(gl[64:128, 512:1024], vp_sb[:, kt, hB, :],
                                         emB[:], start=st, stop=sp,
                                         tile_position=(0, 64),
                                         skip_group_check=True)
                        # denominators: four concurrent M=32 col-tiles
                        nc.tensor.matmul(dn[0:32, :], ones_sb[:], eA[:],
                                         start=st, stop=sp, tile_position=(0, 0))
                        nc.tensor.matmul(dn[32:64, :], ones_sb[:], emA[:],
                                         start=st, stop=sp, tile_position=(0, 32),
                                         skip_group_check=True)
                        nc.tensor.matmul(dn[64:96, :], ones_sb[:], eB[:],
                                         start=st, stop=sp, tile_position=(0, 64),
                                         skip_group_check=True)
                        nc.tensor.matmul(dn[96:128, :], ones_sb[:], emB[:],
                                         start=st, stop=sp, tile_position=(0, 96),
                                         skip_group_check=True)
                        if sp:
                            # unit complete: Ln/Exp of the denom bank issue
                            # right away (frees it); the DMA broadcast + DVE
                            # body defers further into the pipeline
                            tail_ln, tail_rest = make_tail(u, pt, qh, gl, dn)
                            rf = tail_ln()
                            state["rest"] = lambda: tail_rest(rf)

                    return cons

                for kt in range(8):
                    # consumer block for step g-2 goes first: its operands
                    # are ready, keeping the PE FIFO wait-free
                    if len(state["pending"]) >= 2:
                        run_cons()
                    psA = s_ps_pool.tile([128, 512], F32,
                                         name=f"psA_{u}_{kt}", tag="sps")
                    psB = s_ps_pool.tile([128, 512], F32,
                                         name=f"psB_{u}_{kt}", tag="sps")
                    # adj preloads: bf16 identity matmuls (FWL weight loads)
                    nc.tensor.matmul(psA[:], id_sb[:], adt_sb[:, kt, qs],
                                     start=True, stop=False)
                    nc.tensor.matmul(psB[:], id_sb[:], adt_sb[:, kt, qs],
                                     start=True, stop=False)
                    # row-tiled scores: A in array rows 0-63, B in 64-127,
                    # hardware-concurrent into the two banks
                    nc.tensor.matmul(
                        psA[:], kt_sb[0:64, pt, kc(kt)], qt_sb[0:64, pt, qs],
                        start=False, stop=True, tile_position=(0, 0),
                    )
                    nc.tensor.matmul(
                        psB[:], kt_sb[64:128, pt, kc(kt)], qt_sb[64:128, pt, qs],
                        start=False, stop=True, tile_position=(64, 0),
                    )
                    # e = exp(lambda*(S' + adj/lambda)) straight off ACT
                    eA = epool.tile([128, 512], BF16, name=f"eA_{u}_{kt}", tag="e")
                    nc.scalar.activation(eA[:], psA[:], AF.Exp, scale=LAM)
                    eB = epool.tile([128, 512], BF16, name=f"eB_{u}_{kt}", tag="e")
                    nc.scalar.activation(eB[:], psB[:], AF.Exp, scale=LAM)
                    if DEBUG and u == 0 and kt == 0:
                        nc.sync.dma_start(out=dbg["dbg_e"].ap()[:, 0:512], in_=eA[:])
                        nc.sync.dma_start(out=dbg["dbg_e"].ap()[:, 512:1024], in_=eB[:])
                    emA = epool.tile([128, 512], BF16, name=f"emA_{u}_{kt}", tag="e")
                    nc.vector.tensor_tensor(out=emA[:], in0=eA[:],
                                            in1=mt_sb[:, kt, qs], op=ALU.mult)
                    emB = epool.tile([128, 512], BF16, name=f"emB_{u}_{kt}", tag="e")
                    nc.gpsimd.tensor_tensor(out=emB[:], in0=eB[:],
                                            in1=mt_sb[:, kt, qs], op=ALU.mult)
                    state["pending"].append(make_cons(kt, eA, eB, emA, emB))
                    if kt == 1 and state["rest"] is not None:
                        # previous unit's tail body: deferred so its gl-PSUM
                        # reads and broadcast latency hide behind this stream
                        state["rest"]()
                        state["rest"] = None
                    if kt == 3 and len(pieces) > 0:
                        pieces[0]()
                    if kt == 5 and len(pieces) > 1:
                        pieces[1]()

            # V projections first (vp is needed by every unit's AV matmuls),
            # then band 0 Q/K.
            for nt in range(8):
                v_proj(nt)
            qk_piece(wk_sb, xk_sb, bk_sb, kt_sb, 0, 0, riding=False)
            qk_piece(wk_sb, xk_sb, bk_sb, kt_sb, 0, 1, riding=False)
            qk_piece(wq_sb, xq_sb, bq_sb, qt_sb, 0, 0, riding=False)
            qk_piece(wq_sb, xq_sb, bq_sb, qt_sb, 0, 1, riding=False)
            # pair-units; band pt+1 projection pieces ride along (2 per unit)
            for u in range(8):
                pt = u // 2
                pieces = []
                if pt < 3:
                    w_x_b_o = ((wk_sb, xk_sb, bk_sb, kt_sb),
                               (wq_sb, xq_sb, bq_sb, qt_sb))[u % 2]
                    pieces = [
                        (lambda args=w_x_b_o, d=pt + 1, n=nch:
                         qk_piece(*args, d, n)) for nch in range(2)
                    ]
                emit_unit(u, pieces)
            while state["pending"]:
                run_cons()
            state["rest"]()
            if DEBUG:
                for nm, t in (("dbg_qt", qt_sb), ("dbg_kt", kt_sb),
                              ("dbg_tmp", tmp_sb), ("dbg_y", y_sb),
                              ("dbg_wos", wos_sb)):
                    nc.sync.dma_start(
                        out=dbg[nm].ap(),
                        in_=t[:].rearrange("p a n -> p (a n)"))
                nc.sync.dma_start(
                    out=dbg["dbg_vp"].ap(),
                    in_=vp_sb[:].rearrange("p a h d -> p (a h d)"))

        # ---- P4: output projection (partial; host sums pairs + bo) ----
        with tc.tile_pool(name="o_ps", bufs=8, space="PSUM") as o_ps_pool, \
             tc.tile_pool(name="oc", bufs=3) as oc_pool:
            dma_out_engs = (nc.sync, nc.scalar, nc.gpsimd)
            for dot in range(8):
                for qch in range(2):
                    i = dot * 2 + qch
                    ps = o_ps_pool.tile([128, 512], F32,
                                        name=f"ops_{dot}_{qch}", tag="ops")
                    for dvt in range(4):
                        nc.tensor.matmul(
                            ps[:],
                            wos_sb[:, dvt, dot * 128:(dot + 1) * 128],
                            y_sb[:, dvt, qch * 512:(qch + 1) * 512],
                            start=(dvt == 0), stop=(dvt == 3),
                        )
                    ot = oc_pool.tile([128, 512], F32)
                    # PSUM source: only DVE/ACT may read it
                    if i % 2 == 0:
                        nc.vector.tensor_copy(out=ot[:], in_=ps[:])
                    else:
                        nc.scalar.copy(out=ot[:], in_=ps[:])
                    dma_out_engs[i % 3].dma_start(
                        out=out_p.ap()[dot * 128:(dot + 1) * 128,
                                       qch * 512:(qch + 1) * 512],
                        in_=ot[:],
                    )

    nc.finalize()
    return nc


def _get(rho: float):
    key = round(float(rho), 9)
    if key not in _CACHE:
        _CACHE[key] = _build(key)
    return _CACHE[key]


def _make_in_maps(query, key, value, adj, mask, Wq, bq, Wk, bk, Wv, bv, Wo):
    f32 = np.float32
    bf = lambda x: np.ascontiguousarray(np.asarray(x, f32)).astype(ml_dtypes.bfloat16)

    idb = np.eye(128, dtype=f32).astype(ml_dtypes.bfloat16)

    in_maps = []
    for b in range(B):
        xqT = bf(np.asarray(query[b], f32).T)
        xkT = bf(np.asarray(key[b], f32).T)
        xvT = bf(np.asarray(value[b], f32).T)
        adt = bf(np.asarray(adj[b, 0], f32).T * np.float32(ASCALE))
        mtT = bf((np.asarray(mask[b, 0]) != 0).astype(f32).T)
        for g in range(2):
            rows = slice(g * DQL, (g + 1) * DQL)
            in_maps.append({
                "xq": xqT, "xk": xkT, "xv": xvT,
                "wq": bf(np.asarray(Wq, f32)[rows].T),
                "wk": bf(np.asarray(Wk, f32)[rows].T),
                "wv": bf(np.asarray(Wv, f32)[rows].T),
                "bq": np.ascontiguousarray(np.asarray(bq, f32)[rows].reshape(4, 128).T),
                "bk": np.ascontiguousarray(np.asarray(bk, f32)[rows].reshape(4, 128).T),
                "bv": np.ascontiguousarray(np.asarray(bv, f32)[rows].reshape(1, DQL)),
                "adt": adt, "mt": mtT, "idb": idb,
                "wo": bf(np.asarray(Wo, f32)[:, rows].T),
            })
    return in_maps


def kernel(query, key, value, adj, mask, Wq, bq, Wk, bk, Wv, bv, Wo, bo, alpha,
           _want_results=False):
    f32 = np.float32
    a = 1.0 / (1.0 + np.exp(-np.float64(np.asarray(alpha, f32)[0])))
    rho = float(a / (1.0 - a))
    nc = _get(rho)

    in_maps = _make_in_maps(query, key, value, adj, mask,
                            Wq, bq, Wk, bk, Wv, bv, Wo)

    res = run_bass_kernel_spmd(nc, in_maps, list(range(NCORES)), trace=TRACE)
    out = np.empty((B, N, D), f32)
    bo_f = np.asarray(bo, f32)
    for b in range(B):
        out[b] = (res.results[2 * b]["out"] + res.results[2 * b + 1]["out"]).T + bo_f
    if _want_results:
        return out, res
    return out
